# revision 64
# baseline (speedup 1.0000x reference)
"""Trainium2 Bass kernel for nn_Attention_53850299957994 (B=8, S=2048, D=512).

Data-parallel over batch: one batch element per NeuronCore (8 cores).
The host side transposes x/weights into device-friendly bf16 layouts
(pos_table pre-added into x, score scale folded into bq, v-bias folded into
an effective output bias bd + Wd @ bv — exact because softmax rows sum
to 1), runs the SPMD Bass program via concourse, and stacks the per-core
outputs (bf16 wire format, upcast to fp32 on the host).

Device program per core (build_nc), all matmul operands bf16 with fp32 PSUM
accumulation (same PE-array rate as f32r, but FWL weight loads, half the
SBUF/HBM traffic, and 2x DVE on 16-bit ops; ~5.5e-3 relative error):
- x+pos arrives as ready-to-use bf16 projection operands over the sync and
  scalar HWDGE rings; wk/wv/wd ride the separate gpsimd SWDGE lane so the
  first projection matmul starts at ~2us and the scalar queue stays clear
  for activations.
- q/k projections with biases fused into the PSUM->SBUF extraction, done
  entirely as DVE tensor_scalar_add ops (1/sqrt(D) folded into wq on the
  host, so extraction is scale-free and can never queue behind the DMA
  issues that share the scalar engine's ring); the
  attention runs in transposed orientation: scoresT[j,i] tiles with the key
  index on partitions; no max-subtraction (scores ~ N(0,1), exp cannot
  overflow); the j-loop is software-pipelined across i-block boundaries
  (4-deep scores PSUM ring + exp emitted one block ahead of the yd copies)
  so the PE array never waits on the scalar engine's exp or the epilogue.
- BOTH the k- and v-projections are fused away by associativity:
  scores = (x@Wq')(x@Wk')^T = u@x^T with u = x@M, M = Wq'^T@Wk host-
  precomputed (bias cross-terms cancel per softmax row except the per-key
  term, which rides the exp activation bias); and y@Wd^T = (A.x)@(Wd@Wv)^T
  with Wd@Wv host-precomputed. The device runs ONE input projection instead
  of four, skipping 2*S*D*D MACs (~27.3us PE) and 128 matmuls per core.
- softmax denominators accumulate from the exp tiles on the vector engine,
  partition-reduce on gpsimd, then a DVE 32x32 block-transpose +
  diagonal-block extraction puts 1/den on the output partitions; the final
  projection consumes UNNORMALIZED y and the normalize+bias fuse into one
  scalar_tensor_tensor on the PSUM->SBUF store step, so nothing but the yd
  copies sits between the last attention matmul and the final projection.
- wd has its own SBUF slot and the first-column loads have no cross-
  iteration WAR hazards, so in the repeat-loop steady state the next
  iteration's weight/x DMAs prefetch under the current iteration's
  attention tail and the PE runs through the boundary without draining.

Measured 174us harness-protocol slope (R-differencing over 1,17,33),
~142us in the unthrottled R<=17 window (PE-array floor ~137us after both
fusions); TimelineSim single-shot estimate 151us; rel err 4.46e-3.
"""

from contextlib import ExitStack

import ml_dtypes
import numpy as np

import concourse.bacc as bacc
import concourse.bass_isa as bass_isa
import concourse.mybir as mybir
import concourse.tile as tile
from concourse.bass_utils import run_bass_kernel_spmd

P = 128
F32 = mybir.dt.float32
F32R = mybir.dt.float32r
BF16 = mybir.dt.bfloat16


OPTS = frozenset(["expahead", "laststore_sync", "wslots3", "xp_pool", "store_gpsimd", "dent", "nohalf", "expahead2", "out_bf16", "posfold", "startup_lanes", "laststore_alt", "qksplit", "vjsplit", "qkdve", "laststore_sa"])


def build_nc(S=2048, D=512, IB=512, R=1, opts=None):
    opts = OPTS if opts is None else frozenset(opts)
    IB = min(IB, S)
    SC = min(512, S)
    DT = D // P
    ST = S // P
    NB = S // IB
    NSC = S // SC
    JPC = SC // P
    TPB = IB // P          # transpose chunks per i-block (4)
    inv_sqrt_d = 1.0 / float(np.sqrt(D))
    MMDT = BF16

    nc = bacc.Bacc("TRN2", target_bir_lowering=False, debug=False, num_devices=8)

    xT = nc.dram_tensor("xT", [D, S], BF16, kind="ExternalInput").ap()
    xR = nc.dram_tensor("xR", [S, D], BF16, kind="ExternalInput").ap()
    posT = nc.dram_tensor("posT", [D, S], BF16, kind="ExternalInput").ap()
    wqT = nc.dram_tensor("wqT", [D, D], BF16, kind="ExternalInput").ap()
    wkT = nc.dram_tensor("wkT", [D, D], BF16, kind="ExternalInput").ap()
    wvT = nc.dram_tensor("wvT", [D, D], BF16, kind="ExternalInput").ap()
    wdT = nc.dram_tensor("wdT", [D, D], BF16, kind="ExternalInput").ap()
    bqs = nc.dram_tensor("bqs", [D], F32, kind="ExternalInput").ap()
    rb = nc.dram_tensor("rb", [S], F32, kind="ExternalInput").ap()
    bk = nc.dram_tensor("bk", [D], F32, kind="ExternalInput").ap()
    bd = nc.dram_tensor("bd", [D], F32, kind="ExternalInput").ap()  # bd + Wd@bv
    ODT = BF16 if "out_bf16" in opts else F32
    out = nc.dram_tensor("out", [S, D], ODT, kind="ExternalOutput").ap()

    xT_r = xT.rearrange("(o p) s -> p o s", p=P)
    posT_r = posT.rearrange("(o p) s -> p o s", p=P)
    w_r = {
        "q": wqT.rearrange("(o p) e -> p o e", p=P),
        "k": wkT.rearrange("(o p) e -> p o e", p=P),
        "v": wvT.rearrange("(o p) e -> p o e", p=P),
        "d": wdT.rearrange("(o p) e -> p o e", p=P),
    }
    bqs_r = bqs.rearrange("(o p) -> p o", p=P)
    rb_r = rb.rearrange("(t p) -> p t", p=P)
    bk_r = bk.rearrange("(o p) -> p o", p=P)

    with tile.TileContext(nc) as tc, ExitStack() as ctx:
        persist = ctx.enter_context(tc.tile_pool(name="persist", bufs=1))
        pospool = ctx.enter_context(tc.tile_pool(name="pospool", bufs=4))
        expool = ctx.enter_context(tc.tile_pool(name="expool", bufs=6))
        outpool = ctx.enter_context(tc.tile_pool(name="outpool", bufs=4))
        xrpool = ctx.enter_context(tc.tile_pool(name="xrpool", bufs=4))
        psA = ctx.enter_context(tc.tile_pool(name="psA", bufs=4, space="PSUM"))
        psB = ctx.enter_context(tc.tile_pool(name="psB", bufs=4, space="PSUM"))
        denpool = ctx.enter_context(tc.tile_pool(name="denpool", bufs=2))


        def body(_iv=None):
            wt = {}
            engs = (nc.sync, nc.scalar, nc.gpsimd)
            qcur = [0]

            def next_eng():
                e = engs[qcur[0] % 3]
                qcur[0] += 1
                return e

            # q weights reuse slot A (later overwritten by v), k slot B.
            # wd gets its own slot D so a following iteration's wk DMA isn't
            # gated on the final projection still reading slot B.
            if "wslots4" in opts:
                wslot = {"q": "A", "k": "B", "v": "C", "d": "D"}
            elif "wslots3" in opts:
                wslot = {"q": "A", "v": "A", "k": "B", "d": "D"}
            else:
                wslot = {"q": "A", "v": "A", "k": "B", "d": "B"}

            def load_w_o(which, o, eng=None):
                w_t = persist.tile([P, D], MMDT, tag=f"w{wslot[which]}{o}",
                                   name=f"w{which}{o}")
                (eng or next_eng()).dma_start(out=w_t, in_=w_r[which][:, o, :])
                wt.setdefault(which, [None] * DT)[o] = w_t

            def load_w(which):
                for o in range(DT):
                    load_w_o(which, o)

            # streamed x/pos pieces; q-projection chunks follow each column
            xp = {}
            qt = {}
            kt = {}
            vt = {}
            HC = SC // 2  # fast first column: halved pieces, N=256 groups

            def load_x_piece(o, sc, h, W, xeng=None, peng=None, aeng=None):
                lo = sc * SC + h * HC
                xpo = persist.tile([P, W], MMDT, tag=f"xy{o}_{sc}_{h}",
                                   name=f"xp{o}_{sc}_{h}")
                if "posfold" in opts:
                    # pos_table is pre-added into xT on the host; the piece
                    # lands in SBUF ready for the projection matmuls.
                    (xeng or next_eng()).dma_start(out=xpo,
                                                   in_=xT_r[:, o, lo:lo + W])
                    xp[(o, sc, h)] = xpo
                    return
                x_raw = xrpool.tile([P, W], BF16, tag="xr", name="x_raw")
                (xeng or next_eng()).dma_start(out=x_raw,
                                               in_=xT_r[:, o, lo:lo + W])
                pos_p = pospool.tile([P, W], BF16,
                                     tag="pos", name="pos_p")
                (peng or next_eng()).dma_start(out=pos_p,
                                               in_=posT_r[:, o, lo:lo + W])
                (aeng or nc.vector).tensor_add(out=xpo, in0=x_raw, in1=pos_p)
                xp[(o, sc, h)] = xpo

            # startup: first column rides the two HWDGE lanes (alternating so
            # pieces arrive in accumulation order); wk takes the separate
            # SWDGE lane; bulk columns go back to 3-lane round-robin.
            if "startup_lanes" in opts:
                hw = ((nc.sync, nc.gpsimd) if "lanes3" in opts
                      else (nc.sync, nc.scalar))
                W0 = SC if "nohalf" in opts else HC
                for o in range(DT):
                    load_w_o("q", o, eng=hw[o % 2])
                    load_x_piece(o, 0, 0, W0, xeng=hw[(o + 1) % 2],
                                 peng=hw[o % 2])
                    if o == 0:
                        bqs_t = persist.tile([P, DT], F32, tag="bqs")
                        nc.gpsimd.dma_start(out=bqs_t, in_=bqs_r)
                        bk_t = persist.tile([P, DT], F32, tag="bk")
                        nc.gpsimd.dma_start(out=bk_t, in_=bk_r)
                rb_t = persist.tile([P, ST], F32, tag="rb")
                nc.gpsimd.dma_start(out=rb_t, in_=rb_r)
                if "nohalf" not in opts:
                    for o in range(DT):
                        load_x_piece(o, 0, 1, HC, xeng=hw[(o + 1) % 2],
                                     peng=hw[o % 2])
            else:
                # xp adds for the first column go to the pool engine: in the
                # repeat loop it idles through the previous iteration's tail,
                # so the next iteration's first matmul operands are ready
                # before the PE drains.
                xp_aeng = nc.gpsimd if "xp_pool" in opts else None
                W0 = SC if "nohalf" in opts else HC
                for o in range(DT):
                    load_w_o("q", o)
                    load_x_piece(o, 0, 0, W0, aeng=xp_aeng)
                    if o == 0:
                        bqs_t = persist.tile([P, DT], F32, tag="bqs")
                        nc.gpsimd.dma_start(out=bqs_t, in_=bqs_r)
                        bk_t = persist.tile([P, DT], F32, tag="bk")
                        nc.gpsimd.dma_start(out=bk_t, in_=bk_r)

            def proj(which, sc, h):
                # with qksplit the 1/sqrt(D) is folded into wqT on the host,
                # so extraction is scale-free and alternates between the
                # scalar activation and a DVE tensor_scalar_add, halving the
                # PSUM-release latency per column.
                qks = "qksplit" in opts
                dst, b_t, scl = (
                    (qt, bqs_t, 1.0 if qks else inv_sqrt_d) if which == "q"
                    else (kt, bk_t, 1.0))
                halved = sc == 0 and "nohalf" not in opts
                W = HC if halved else SC
                for et in range(DT):
                    ps = psA.tile([P, SC], F32, tag="A")
                    for o in range(DT):
                        nc.tensor.matmul(
                            ps[:, h * HC:h * HC + W] if halved else ps,
                            wt[which][o][:, et * P:(et + 1) * P],
                            xp[(o, sc, h)],
                            start=(o == 0),
                            stop=(o == DT - 1),
                        )
                    if (et, sc) not in dst:
                        ch = persist.tile(
                            [P, SC], MMDT, tag=f"{which}{et}_{sc}",
                            name=f"{which}{et}_{sc}")
                        dst[(et, sc)] = ch
                    dst_ap = (dst[(et, sc)][:, h * HC:h * HC + W]
                              if halved else dst[(et, sc)])
                    ps_ap = ps[:, h * HC:h * HC + W] if halved else ps
                    if qks and (et % 2 == 1 or "qkdve" in opts):
                        nc.vector.tensor_scalar_add(
                            out=dst_ap, in0=ps_ap,
                            scalar1=b_t[:, et:et + 1])
                    else:
                        nc.scalar.activation(
                            out=dst_ap, in_=ps_ap,
                            func=mybir.ActivationFunctionType.Identity,
                            bias=b_t[:, et:et + 1], scale=scl,
                        )

            for sc in range(NSC):
                halves = ((0, 1) if sc == 0 and "nohalf" not in opts
                          else (0,))
                for h in halves:
                    if sc > 0:
                        for o in range(DT):
                            load_x_piece(
                                o, sc, h, SC,
                                xeng=((nc.sync, nc.gpsimd)[o % 2]
                                      if "lanes3" in opts else None))
                    elif h == 1 and "startup_lanes" not in opts:
                        for o in range(DT):
                            load_x_piece(o, 0, 1, HC)
                    if sc == 0 and h == 0 and "startup_lanes" not in opts:
                        load_w("k")
                    proj("q", sc, h)

            bd_bc = persist.tile([P, D], F32, tag="bd_bc")
            nc.gpsimd.dma_start(out=bd_bc, in_=bd.unsqueeze(0).to_broadcast((P, D)))

            sps_q = {}   # (ib, jt) -> psum

            def emit_scores(ib, jt):
                # k-projection fused away the same way as v:
                #   scores = (x@Wq')(x@Wk')^T = u @ x^T with u = x@M,
                #   M = Wq'^T@Wk precomputed on the host (in wqT's slot);
                # the bias cross-terms cancel per softmax row except the
                # per-key term, which rides the exp activation bias (rb).
                sps = psB.tile([P, IB], F32, tag="B", name=f"sps{ib}_{jt}")
                for o in range(DT):
                    nc.tensor.matmul(
                        sps,
                        xp[(o, jt // JPC, 0)][:,
                            (jt % JPC) * P:(jt % JPC + 1) * P],
                        qt[(o, ib)],
                        start=(o == 0),
                        stop=(o == DT - 1),
                    )
                sps_q[(ib, jt)] = sps

            emitted = set()

            def emit_next(ib, jt):
                if (ib, jt) not in emitted and ib < NB:
                    emitted.add((ib, jt))
                    emit_scores(ib, jt)

            # v-projection fused away by associativity:
            #   y@Wd^T = (A.(x@Wv^T))@Wd^T = (A.x)@(Wd@Wv)^T
            # so attention multiplies the raw x rows (host supplies them
            # untransposed, pos pre-added) and the final projection uses the
            # host-precomputed Wd@Wv. Saves S*D*D MACs (~13.7us PE) per core.
            for o in range(DT):
                load_w_o("d", o)
            for jt in range(ST):
                vj = persist.tile([P, D], MMDT, tag=f"v{jt}", name=f"v{jt}")
                next_eng().dma_start(out=vj, in_=xR[jt * P:(jt + 1) * P, :])
                vt[jt] = vj

            ex_q = {}

            def emit_exp(ib, jt):
                if (ib, jt) not in ex_q:
                    ex = expool.tile([P, IB], MMDT, tag="exp")
                    nc.scalar.activation(
                        out=ex, in_=sps_q.pop((ib, jt)),
                        func=mybir.ActivationFunctionType.Exp,
                        bias=rb_t[:, jt:jt + 1],
                    )
                    ex_q[(ib, jt)] = ex

            # attention + per-ib denominator transpose + final projection
            emit_next(0, 0)
            emit_exp(0, 0)
            emit_next(0, 1)
            for ib in range(NB):
                i0 = ib * IB
                yps = [psA.tile([P, IB], F32, tag="A", name=f"yps{dc}")
                       for dc in range(DT)]
                acc = denpool.tile([P, IB], F32, tag="acc")
                for jt in range(ST):
                    if jt + 1 < ST:
                        emit_next(ib, jt + 1)
                    elif ib + 1 < NB:
                        emit_next(ib + 1, 0)  # keep PE fed across the block edge
                    emit_exp(ib, jt)
                    ex = ex_q.pop((ib, jt))
                    for dc in range(DT):
                        nc.tensor.matmul(
                            yps[dc],
                            vt[jt][:, dc * P:(dc + 1) * P],
                            ex,
                            start=(jt == 0),
                            stop=(jt == ST - 1),
                        )
                    if jt == 0:
                        nc.vector.tensor_copy(out=acc, in_=ex)
                    else:
                        nc.vector.tensor_add(out=acc, in0=acc, in1=ex)

                if ib + 1 < NB and "expahead" in opts:
                    emit_exp(ib + 1, 0)  # ahead of the yd copies on ACT
                yd = []
                for dc in range(DT):
                    ydt = persist.tile([P, IB], MMDT, tag=f"yd{dc}",
                                       name=f"y{dc}_{ib}", bufs=2)
                    if dc % 2 == 0:
                        nc.vector.tensor_copy(out=ydt, in_=yps[dc])
                    else:
                        nc.scalar.activation(
                            out=ydt, in_=yps[dc],
                            func=mybir.ActivationFunctionType.Identity)
                    yd.append(ydt)
                denrep = denpool.tile([P, IB], F32, tag="denrep")
                nc.gpsimd.partition_all_reduce(
                    denrep, acc, channels=P, reduce_op=bass_isa.ReduceOp.add)
                if "dent" in opts:
                    # transpose the replicated denominator onto partitions
                    # (diagonal 32-blocks of the DVE block-transpose), so the
                    # final projection consumes unnormalized y and the
                    # normalize+bias fuse into one scalar_tensor_tensor on
                    # the PSUM->SBUF step, off the fps critical path.
                    Tr = denpool.tile([P, IB], F32, tag="Tr")
                    nc.vector.transpose(out=Tr, in_=denrep)
                    denT = denpool.tile([P, TPB], F32, tag="dT")
                    for ii in range(TPB):
                        for pb in range(4):
                            nc.vector.tensor_copy(
                                out=denT[32 * pb:32 * pb + 32, ii:ii + 1],
                                in_=Tr[32 * pb:32 * pb + 32,
                                       ii * P + 32 * pb:ii * P + 32 * pb + 1])
                    rT = denpool.tile([P, TPB], F32, tag="rT")
                    nc.vector.reciprocal(out=rT, in_=denT)
                else:
                    rrep = denpool.tile([P, IB], F32, tag="rrep")
                    nc.vector.reciprocal(out=rrep, in_=denrep)
                if ib + 1 < NB:
                    emit_next(ib + 1, 1)  # PE work before the fps block
                    if "expahead2" in opts:
                        emit_exp(ib + 1, 1)
                if "dent" not in opts:
                    for dc in range(DT):
                        # split the normalize across DVE and gpsimd so the fps
                        # matmuls aren't serialized behind one engine
                        eng = (nc.gpsimd if dc % 2 == 1 and "normsplit" in opts
                               else nc.vector)
                        eng.tensor_tensor(
                            out=yd[dc], in0=yd[dc], in1=rrep,
                            op=mybir.AluOpType.mult)

                for ii in range(TPB):
                    it = ib * TPB + ii
                    fps = psB.tile([P, D], F32, tag="B")
                    for o in range(DT):
                        nc.tensor.matmul(
                            fps,
                            yd[o][:, ii * P:(ii + 1) * P],
                            wt["d"][o],
                            start=(o == 0),
                            stop=(o == DT - 1),
                        )
                    f_sb = outpool.tile([P, D], ODT, tag="fout")
                    mid_eng = (nc.gpsimd if "store_gpsimd" in opts
                               else nc.scalar)
                    if ib == NB - 1 and "laststore_sa" in opts:
                        # keep the sync queue free of iteration-tail stores so
                        # the next repeat iteration's x/wq prefetch is never
                        # queued behind them; ACT has no early next-iteration
                        # work (extractions live on DVE now).
                        seng = nc.scalar if it % 2 == 0 else nc.gpsimd
                    elif ib == NB - 1 and "laststore_alt" in opts:
                        seng = nc.scalar if it % 2 == 0 else nc.sync
                    elif ib == NB - 1 and "laststore_sync" in opts:
                        seng = nc.sync
                    else:
                        seng = mid_eng if it % 2 == 0 else nc.sync
                    tail_split = (ib == NB - 1 and "tailsplit" in opts)
                    hs = (0, D // 2) if tail_split else (0,)
                    wd_ = D // 2 if tail_split else D
                    for lo in hs:
                        if "dent" in opts:
                            nc.vector.scalar_tensor_tensor(
                                out=f_sb[:, lo:lo + wd_],
                                in0=fps[:, lo:lo + wd_],
                                scalar=rT[:, ii:ii + 1],
                                in1=bd_bc[:, lo:lo + wd_],
                                op0=mybir.AluOpType.mult,
                                op1=mybir.AluOpType.add)
                        else:
                            nc.vector.tensor_add(
                                out=f_sb[:, lo:lo + wd_],
                                in0=fps[:, lo:lo + wd_],
                                in1=bd_bc[:, lo:lo + wd_])
                        seng.dma_start(
                            out=out[it * P:(it + 1) * P, lo:lo + wd_],
                            in_=f_sb[:, lo:lo + wd_])
                if ib + 1 < NB:
                    emit_next(ib + 1, 2)  # more PE runahead over the edge

        if R == 1:
            body()
        else:
            with tc.For_i(0, R, 1, hint_engines=(
                    mybir.EngineType.PE, mybir.EngineType.Activation,
                    mybir.EngineType.DVE)) as iv:
                body(iv)

    nc.compile()
    return nc


def host_prep(x, pos_table, Wq, bq, Wk, bk, Wv, bv, Wd, bd):
    B, S, D = x.shape
    f = np.float32
    bf = ml_dtypes.bfloat16
    shared = {
        "posT": np.ascontiguousarray(
            np.asarray(pos_table, dtype=f)[:S].T).astype(bf),
        "wqT": np.ascontiguousarray(
            (np.asarray(Wq, dtype=f).T / np.sqrt(np.float32(D)))
            @ np.asarray(Wk, dtype=f)).astype(bf),
        "wkT": np.ascontiguousarray(np.asarray(Wk, dtype=f).T).astype(bf),
        "wvT": np.ascontiguousarray(np.asarray(Wv, dtype=f).T).astype(bf),
        "wdT": np.ascontiguousarray(
            (np.asarray(Wd, dtype=f) @ np.asarray(Wv, dtype=f)).T).astype(bf),
        "bqs": np.zeros_like(np.asarray(bq, dtype=f)),
        "bk": np.asarray(bk, dtype=f),
        "bd": (np.asarray(bd, dtype=f)
               + np.asarray(Wd, dtype=f) @ np.asarray(bv, dtype=f)),
    }
    posT_f = np.asarray(pos_table, dtype=f)[:S].T
    in_maps = []
    for b in range(B):
        m = dict(shared)
        if "posfold" in OPTS:
            xpb = np.asarray(x[b], dtype=f) + posT_f.T
        else:
            xpb = np.asarray(x[b], dtype=f)
        m["xT"] = np.ascontiguousarray(xpb.T).astype(bf)
        m["xR"] = np.ascontiguousarray(xpb).astype(bf)
        m["rb"] = (xpb @ (np.asarray(Wk, dtype=f).T
                          @ (np.asarray(bq, dtype=f)
                             / np.sqrt(np.float32(D))))).astype(f)
        in_maps.append(m)
    return in_maps


_NC_CACHE = {}


def _get_nc(S, D, R=1):
    key = (S, D, R)
    if key not in _NC_CACHE:
        _NC_CACHE[key] = build_nc(S=S, D=D, R=R)
    return _NC_CACHE[key]


def kernel(x, pos_table, Wq, bq, Wk, bk, Wv, bv, Wd, bd):
    """Full inputs -> full output [B, S, D], computed on 8 NeuronCores."""
    x = np.asarray(x)
    B, S, D = x.shape
    assert B == 8, f"expected B=8, got {B}"
    nc = _get_nc(S, D)
    in_maps = host_prep(x, np.asarray(pos_table), np.asarray(Wq),
                        np.asarray(bq), np.asarray(Wk), np.asarray(bk),
                        np.asarray(Wv), np.asarray(bv), np.asarray(Wd),
                        np.asarray(bd))
    res = run_bass_kernel_spmd(nc, in_maps, core_ids=list(range(B)))
    return np.stack([np.asarray(res.results[b]["out"], dtype=np.float32)
                     for b in range(B)])



# revision 65
# speedup vs baseline: 1.0620x; 1.0620x over previous
"""Trainium2 Bass kernel for nn_Attention_53850299957994 (B=8, S=2048, D=512).

Data-parallel over batch: one batch element per NeuronCore (8 cores).
The host side transposes x/weights into device-friendly bf16 layouts
(pos_table pre-added into x, score scale folded into bq, v-bias folded into
an effective output bias bd + Wd @ bv — exact because softmax rows sum
to 1), runs the SPMD Bass program via concourse, and stacks the per-core
outputs (bf16 wire format, upcast to fp32 on the host).

Device program per core (build_nc), all matmul operands bf16 with fp32 PSUM
accumulation (same PE-array rate as f32r, but FWL weight loads, half the
SBUF/HBM traffic, and 2x DVE on 16-bit ops; ~5.5e-3 relative error):
- x+pos arrives as ready-to-use bf16 projection operands over the sync and
  scalar HWDGE rings; wk/wv/wd ride the separate gpsimd SWDGE lane so the
  first projection matmul starts at ~2us and the scalar queue stays clear
  for activations.
- q/k projections with biases fused into the PSUM->SBUF extraction, done
  entirely as DVE tensor_scalar_add ops (1/sqrt(D) folded into wq on the
  host, so extraction is scale-free and can never queue behind the DMA
  issues that share the scalar engine's ring); the
  attention runs in transposed orientation: scoresT[j,i] tiles with the key
  index on partitions; no max-subtraction (scores ~ N(0,1), exp cannot
  overflow); the j-loop is software-pipelined across i-block boundaries
  (4-deep scores PSUM ring + exp emitted one block ahead of the yd copies)
  so the PE array never waits on the scalar engine's exp or the epilogue.
- BOTH the k- and v-projections are fused away by associativity:
  scores = (x@Wq')(x@Wk')^T = u@x^T with u = x@M, M = Wq'^T@Wk host-
  precomputed (bias cross-terms cancel per softmax row except the per-key
  term, which rides the exp activation bias); and y@Wd^T = (A.x)@(Wd@Wv)^T
  with Wd@Wv host-precomputed. The device runs ONE input projection instead
  of four, skipping 2*S*D*D MACs (~27.3us PE) and 128 matmuls per core.
- softmax denominators accumulate from the exp tiles on the vector engine,
  partition-reduce on gpsimd, then a DVE 32x32 block-transpose +
  diagonal-block extraction puts 1/den on the output partitions; the final
  projection consumes UNNORMALIZED y and the normalize+bias fuse into one
  scalar_tensor_tensor on the PSUM->SBUF store step, so nothing but the yd
  copies sits between the last attention matmul and the final projection.
- wd has its own SBUF slot and the first-column loads have no cross-
  iteration WAR hazards, so in the repeat-loop steady state the next
  iteration's weight/x DMAs prefetch under the current iteration's
  attention tail and the PE runs through the boundary without draining.

Measured 174us harness-protocol slope (R-differencing over 1,17,33),
~142us in the unthrottled R<=17 window (PE-array floor ~137us after both
fusions); TimelineSim single-shot estimate 151us; rel err 4.46e-3.
"""

from contextlib import ExitStack

import ml_dtypes
import numpy as np

import concourse.bacc as bacc
import concourse.bass_isa as bass_isa
import concourse.mybir as mybir
import concourse.tile as tile
from concourse.bass_utils import run_bass_kernel_spmd

P = 128
F32 = mybir.dt.float32
F32R = mybir.dt.float32r
BF16 = mybir.dt.bfloat16


OPTS = frozenset(["expahead", "laststore_sync", "wslots3", "xp_pool", "store_gpsimd", "dent", "nohalf", "expahead2", "out_bf16", "posfold", "startup_lanes", "laststore_alt", "qksplit", "vjsplit", "qkdve", "laststore_sa"])


def build_nc(S=2048, D=512, IB=512, R=1, opts=None):
    opts = OPTS if opts is None else frozenset(opts)
    IB = min(IB, S)
    SC = min(512, S)
    DT = D // P
    ST = S // P
    NB = S // IB
    NSC = S // SC
    JPC = SC // P
    TPB = IB // P          # transpose chunks per i-block (4)
    inv_sqrt_d = 1.0 / float(np.sqrt(D))
    MMDT = BF16

    nc = bacc.Bacc("TRN2", target_bir_lowering=False, debug=False, num_devices=8)

    xT = nc.dram_tensor("xT", [D, S], BF16, kind="ExternalInput").ap()
    xR = nc.dram_tensor("xR", [S, D], BF16, kind="ExternalInput").ap()
    posT = nc.dram_tensor("posT", [D, S], BF16, kind="ExternalInput").ap()
    wqT = nc.dram_tensor("wqT", [D, D], BF16, kind="ExternalInput").ap()
    wkT = nc.dram_tensor("wkT", [D, D], BF16, kind="ExternalInput").ap()
    wvT = nc.dram_tensor("wvT", [D, D], BF16, kind="ExternalInput").ap()
    wdT = nc.dram_tensor("wdT", [D, D], BF16, kind="ExternalInput").ap()
    bqs = nc.dram_tensor("bqs", [D], F32, kind="ExternalInput").ap()
    rb = nc.dram_tensor("rb", [S], F32, kind="ExternalInput").ap()
    bk = nc.dram_tensor("bk", [D], F32, kind="ExternalInput").ap()
    bd = nc.dram_tensor("bd", [D], F32, kind="ExternalInput").ap()  # bd + Wd@bv
    ODT = BF16 if "out_bf16" in opts else F32
    out = nc.dram_tensor("out", [S, D], ODT, kind="ExternalOutput").ap()

    xT_r = xT.rearrange("(o p) s -> p o s", p=P)
    posT_r = posT.rearrange("(o p) s -> p o s", p=P)
    w_r = {
        "q": wqT.rearrange("(o p) e -> p o e", p=P),
        "k": wkT.rearrange("(o p) e -> p o e", p=P),
        "v": wvT.rearrange("(o p) e -> p o e", p=P),
        "d": wdT.rearrange("(o p) e -> p o e", p=P),
    }
    bqs_r = bqs.rearrange("(o p) -> p o", p=P)
    rb_r = rb.rearrange("(t p) -> p t", p=P)
    bk_r = bk.rearrange("(o p) -> p o", p=P)

    with tile.TileContext(nc) as tc, ExitStack() as ctx:
        persist = ctx.enter_context(tc.tile_pool(name="persist", bufs=1))
        pospool = ctx.enter_context(tc.tile_pool(name="pospool", bufs=4))
        expool = ctx.enter_context(tc.tile_pool(name="expool", bufs=6))
        outpool = ctx.enter_context(tc.tile_pool(name="outpool", bufs=4))
        xrpool = ctx.enter_context(tc.tile_pool(name="xrpool", bufs=4))
        psA = ctx.enter_context(tc.tile_pool(name="psA", bufs=4, space="PSUM"))
        psB = ctx.enter_context(tc.tile_pool(name="psB", bufs=4, space="PSUM"))
        denpool = ctx.enter_context(tc.tile_pool(name="denpool", bufs=2))


        def body(_iv=None):
            wt = {}
            engs = (nc.sync, nc.scalar, nc.gpsimd)
            qcur = [0]

            def next_eng():
                e = engs[qcur[0] % 3]
                qcur[0] += 1
                return e

            # q weights reuse slot A (later overwritten by v), k slot B.
            # wd gets its own slot D so a following iteration's wk DMA isn't
            # gated on the final projection still reading slot B.
            if "wslots4" in opts:
                wslot = {"q": "A", "k": "B", "v": "C", "d": "D"}
            elif "wslots3" in opts:
                wslot = {"q": "A", "v": "A", "k": "B", "d": "D"}
            else:
                wslot = {"q": "A", "v": "A", "k": "B", "d": "B"}

            def load_w_o(which, o, eng=None):
                w_t = persist.tile([P, D], MMDT, tag=f"w{wslot[which]}{o}",
                                   name=f"w{which}{o}")
                (eng or next_eng()).dma_start(out=w_t, in_=w_r[which][:, o, :])
                wt.setdefault(which, [None] * DT)[o] = w_t

            def load_w(which):
                for o in range(DT):
                    load_w_o(which, o)

            # streamed x/pos pieces; q-projection chunks follow each column
            xp = {}
            qt = {}
            kt = {}
            vt = {}
            HC = SC // 2  # fast first column: halved pieces, N=256 groups

            def load_x_piece(o, sc, h, W, xeng=None, peng=None, aeng=None):
                lo = sc * SC + h * HC
                xpo = persist.tile([P, W], MMDT, tag=f"xy{o}_{sc}_{h}",
                                   name=f"xp{o}_{sc}_{h}", bufs=2)
                if "posfold" in opts:
                    # pos_table is pre-added into xT on the host; the piece
                    # lands in SBUF ready for the projection matmuls.
                    (xeng or next_eng()).dma_start(out=xpo,
                                                   in_=xT_r[:, o, lo:lo + W])
                    xp[(o, sc, h)] = xpo
                    return
                x_raw = xrpool.tile([P, W], BF16, tag="xr", name="x_raw")
                (xeng or next_eng()).dma_start(out=x_raw,
                                               in_=xT_r[:, o, lo:lo + W])
                pos_p = pospool.tile([P, W], BF16,
                                     tag="pos", name="pos_p")
                (peng or next_eng()).dma_start(out=pos_p,
                                               in_=posT_r[:, o, lo:lo + W])
                (aeng or nc.vector).tensor_add(out=xpo, in0=x_raw, in1=pos_p)
                xp[(o, sc, h)] = xpo

            # startup: first column rides the two HWDGE lanes (alternating so
            # pieces arrive in accumulation order); wk takes the separate
            # SWDGE lane; bulk columns go back to 3-lane round-robin.
            if "startup_lanes" in opts:
                hw = ((nc.sync, nc.gpsimd) if "lanes3" in opts
                      else (nc.sync, nc.scalar))
                W0 = SC if "nohalf" in opts else HC
                for o in range(DT):
                    load_w_o("q", o, eng=hw[o % 2])
                    load_x_piece(o, 0, 0, W0, xeng=hw[(o + 1) % 2],
                                 peng=hw[o % 2])
                    if o == 0:
                        bqs_t = persist.tile([P, DT], F32, tag="bqs")
                        nc.gpsimd.dma_start(out=bqs_t, in_=bqs_r)
                        bk_t = persist.tile([P, DT], F32, tag="bk")
                        nc.gpsimd.dma_start(out=bk_t, in_=bk_r)
                rb_t = persist.tile([P, ST], F32, tag="rb")
                nc.gpsimd.dma_start(out=rb_t, in_=rb_r)
                if "nohalf" not in opts:
                    for o in range(DT):
                        load_x_piece(o, 0, 1, HC, xeng=hw[(o + 1) % 2],
                                     peng=hw[o % 2])
            else:
                # xp adds for the first column go to the pool engine: in the
                # repeat loop it idles through the previous iteration's tail,
                # so the next iteration's first matmul operands are ready
                # before the PE drains.
                xp_aeng = nc.gpsimd if "xp_pool" in opts else None
                W0 = SC if "nohalf" in opts else HC
                for o in range(DT):
                    load_w_o("q", o)
                    load_x_piece(o, 0, 0, W0, aeng=xp_aeng)
                    if o == 0:
                        bqs_t = persist.tile([P, DT], F32, tag="bqs")
                        nc.gpsimd.dma_start(out=bqs_t, in_=bqs_r)
                        bk_t = persist.tile([P, DT], F32, tag="bk")
                        nc.gpsimd.dma_start(out=bk_t, in_=bk_r)

            def proj(which, sc, h):
                # with qksplit the 1/sqrt(D) is folded into wqT on the host,
                # so extraction is scale-free and alternates between the
                # scalar activation and a DVE tensor_scalar_add, halving the
                # PSUM-release latency per column.
                qks = "qksplit" in opts
                dst, b_t, scl = (
                    (qt, bqs_t, 1.0 if qks else inv_sqrt_d) if which == "q"
                    else (kt, bk_t, 1.0))
                halved = sc == 0 and "nohalf" not in opts
                W = HC if halved else SC
                for et in range(DT):
                    ps = psA.tile([P, SC], F32, tag="A")
                    for o in range(DT):
                        nc.tensor.matmul(
                            ps[:, h * HC:h * HC + W] if halved else ps,
                            wt[which][o][:, et * P:(et + 1) * P],
                            xp[(o, sc, h)],
                            start=(o == 0),
                            stop=(o == DT - 1),
                        )
                    if (et, sc) not in dst:
                        ch = persist.tile(
                            [P, SC], MMDT, tag=f"{which}{et}_{sc}",
                            name=f"{which}{et}_{sc}", bufs=2)
                        dst[(et, sc)] = ch
                    dst_ap = (dst[(et, sc)][:, h * HC:h * HC + W]
                              if halved else dst[(et, sc)])
                    ps_ap = ps[:, h * HC:h * HC + W] if halved else ps
                    if qks and (et % 2 == 1 or "qkdve" in opts):
                        nc.vector.tensor_scalar_add(
                            out=dst_ap, in0=ps_ap,
                            scalar1=b_t[:, et:et + 1])
                    else:
                        nc.scalar.activation(
                            out=dst_ap, in_=ps_ap,
                            func=mybir.ActivationFunctionType.Identity,
                            bias=b_t[:, et:et + 1], scale=scl,
                        )

            for sc in range(NSC):
                halves = ((0, 1) if sc == 0 and "nohalf" not in opts
                          else (0,))
                for h in halves:
                    if sc > 0:
                        for o in range(DT):
                            load_x_piece(
                                o, sc, h, SC,
                                xeng=((nc.sync, nc.gpsimd)[o % 2]
                                      if "lanes3" in opts else None))
                    elif h == 1 and "startup_lanes" not in opts:
                        for o in range(DT):
                            load_x_piece(o, 0, 1, HC)
                    if sc == 0 and h == 0 and "startup_lanes" not in opts:
                        load_w("k")
                    proj("q", sc, h)

            bd_bc = persist.tile([P, D], F32, tag="bd_bc")
            nc.gpsimd.dma_start(out=bd_bc, in_=bd.unsqueeze(0).to_broadcast((P, D)))

            sps_q = {}   # (ib, jt) -> psum

            def emit_scores(ib, jt):
                # k-projection fused away the same way as v:
                #   scores = (x@Wq')(x@Wk')^T = u @ x^T with u = x@M,
                #   M = Wq'^T@Wk precomputed on the host (in wqT's slot);
                # the bias cross-terms cancel per softmax row except the
                # per-key term, which rides the exp activation bias (rb).
                sps = psB.tile([P, IB], F32, tag="B", name=f"sps{ib}_{jt}")
                for o in range(DT):
                    nc.tensor.matmul(
                        sps,
                        xp[(o, jt // JPC, 0)][:,
                            (jt % JPC) * P:(jt % JPC + 1) * P],
                        qt[(o, ib)],
                        start=(o == 0),
                        stop=(o == DT - 1),
                    )
                sps_q[(ib, jt)] = sps

            emitted = set()

            def emit_next(ib, jt):
                if (ib, jt) not in emitted and ib < NB:
                    emitted.add((ib, jt))
                    emit_scores(ib, jt)

            # v-projection fused away by associativity:
            #   y@Wd^T = (A.(x@Wv^T))@Wd^T = (A.x)@(Wd@Wv)^T
            # so attention multiplies the raw x rows (host supplies them
            # untransposed, pos pre-added) and the final projection uses the
            # host-precomputed Wd@Wv. Saves S*D*D MACs (~13.7us PE) per core.
            for o in range(DT):
                load_w_o("d", o)
            for jt in range(ST):
                vj = persist.tile([P, D], MMDT, tag=f"v{jt}", name=f"v{jt}",
                                  bufs=2)
                next_eng().dma_start(out=vj, in_=xR[jt * P:(jt + 1) * P, :])
                vt[jt] = vj

            ex_q = {}

            def emit_exp(ib, jt):
                if (ib, jt) not in ex_q:
                    ex = expool.tile([P, IB], MMDT, tag="exp")
                    nc.scalar.activation(
                        out=ex, in_=sps_q.pop((ib, jt)),
                        func=mybir.ActivationFunctionType.Exp,
                        bias=rb_t[:, jt:jt + 1],
                    )
                    ex_q[(ib, jt)] = ex

            # attention + per-ib denominator transpose + final projection
            emit_next(0, 0)
            emit_exp(0, 0)
            emit_next(0, 1)
            for ib in range(NB):
                i0 = ib * IB
                yps = [psA.tile([P, IB], F32, tag="A", name=f"yps{dc}")
                       for dc in range(DT)]
                acc = denpool.tile([P, IB], F32, tag="acc")
                for jt in range(ST):
                    if jt + 1 < ST:
                        emit_next(ib, jt + 1)
                    elif ib + 1 < NB:
                        emit_next(ib + 1, 0)  # keep PE fed across the block edge
                    emit_exp(ib, jt)
                    ex = ex_q.pop((ib, jt))
                    for dc in range(DT):
                        nc.tensor.matmul(
                            yps[dc],
                            vt[jt][:, dc * P:(dc + 1) * P],
                            ex,
                            start=(jt == 0),
                            stop=(jt == ST - 1),
                        )
                    if jt == 0:
                        nc.vector.tensor_copy(out=acc, in_=ex)
                    else:
                        nc.vector.tensor_add(out=acc, in0=acc, in1=ex)

                if ib + 1 < NB and "expahead" in opts:
                    emit_exp(ib + 1, 0)  # ahead of the yd copies on ACT
                yd = []
                for dc in range(DT):
                    ydt = persist.tile([P, IB], MMDT, tag=f"yd{dc}",
                                       name=f"y{dc}_{ib}", bufs=2)
                    if dc % 2 == 0:
                        nc.vector.tensor_copy(out=ydt, in_=yps[dc])
                    else:
                        nc.scalar.activation(
                            out=ydt, in_=yps[dc],
                            func=mybir.ActivationFunctionType.Identity)
                    yd.append(ydt)
                denrep = denpool.tile([P, IB], F32, tag="denrep")
                nc.gpsimd.partition_all_reduce(
                    denrep, acc, channels=P, reduce_op=bass_isa.ReduceOp.add)
                if "dent" in opts:
                    # transpose the replicated denominator onto partitions
                    # (diagonal 32-blocks of the DVE block-transpose), so the
                    # final projection consumes unnormalized y and the
                    # normalize+bias fuse into one scalar_tensor_tensor on
                    # the PSUM->SBUF step, off the fps critical path.
                    Tr = denpool.tile([P, IB], F32, tag="Tr")
                    nc.vector.transpose(out=Tr, in_=denrep)
                    denT = denpool.tile([P, TPB], F32, tag="dT")
                    for ii in range(TPB):
                        for pb in range(4):
                            nc.vector.tensor_copy(
                                out=denT[32 * pb:32 * pb + 32, ii:ii + 1],
                                in_=Tr[32 * pb:32 * pb + 32,
                                       ii * P + 32 * pb:ii * P + 32 * pb + 1])
                    rT = denpool.tile([P, TPB], F32, tag="rT")
                    nc.vector.reciprocal(out=rT, in_=denT)
                else:
                    rrep = denpool.tile([P, IB], F32, tag="rrep")
                    nc.vector.reciprocal(out=rrep, in_=denrep)
                if ib + 1 < NB:
                    emit_next(ib + 1, 1)  # PE work before the fps block
                    if "expahead2" in opts:
                        emit_exp(ib + 1, 1)
                if "dent" not in opts:
                    for dc in range(DT):
                        # split the normalize across DVE and gpsimd so the fps
                        # matmuls aren't serialized behind one engine
                        eng = (nc.gpsimd if dc % 2 == 1 and "normsplit" in opts
                               else nc.vector)
                        eng.tensor_tensor(
                            out=yd[dc], in0=yd[dc], in1=rrep,
                            op=mybir.AluOpType.mult)

                for ii in range(TPB):
                    it = ib * TPB + ii
                    fps = psB.tile([P, D], F32, tag="B")
                    for o in range(DT):
                        nc.tensor.matmul(
                            fps,
                            yd[o][:, ii * P:(ii + 1) * P],
                            wt["d"][o],
                            start=(o == 0),
                            stop=(o == DT - 1),
                        )
                    f_sb = outpool.tile([P, D], ODT, tag="fout")
                    mid_eng = (nc.gpsimd if "store_gpsimd" in opts
                               else nc.scalar)
                    if ib == NB - 1 and "laststore_sa" in opts:
                        # keep the sync queue free of iteration-tail stores so
                        # the next repeat iteration's x/wq prefetch is never
                        # queued behind them; ACT has no early next-iteration
                        # work (extractions live on DVE now).
                        seng = nc.scalar if it % 2 == 0 else nc.gpsimd
                    elif ib == NB - 1 and "laststore_alt" in opts:
                        seng = nc.scalar if it % 2 == 0 else nc.sync
                    elif ib == NB - 1 and "laststore_sync" in opts:
                        seng = nc.sync
                    else:
                        seng = mid_eng if it % 2 == 0 else nc.sync
                    tail_split = (ib == NB - 1 and "tailsplit" in opts)
                    hs = (0, D // 2) if tail_split else (0,)
                    wd_ = D // 2 if tail_split else D
                    for lo in hs:
                        if "dent" in opts:
                            nc.vector.scalar_tensor_tensor(
                                out=f_sb[:, lo:lo + wd_],
                                in0=fps[:, lo:lo + wd_],
                                scalar=rT[:, ii:ii + 1],
                                in1=bd_bc[:, lo:lo + wd_],
                                op0=mybir.AluOpType.mult,
                                op1=mybir.AluOpType.add)
                        else:
                            nc.vector.tensor_add(
                                out=f_sb[:, lo:lo + wd_],
                                in0=fps[:, lo:lo + wd_],
                                in1=bd_bc[:, lo:lo + wd_])
                        seng.dma_start(
                            out=out[it * P:(it + 1) * P, lo:lo + wd_],
                            in_=f_sb[:, lo:lo + wd_])
                if ib + 1 < NB:
                    emit_next(ib + 1, 2)  # more PE runahead over the edge

        if R == 1:
            body()
        else:
            with tc.For_i(0, R, 1, hint_engines=(
                    mybir.EngineType.PE, mybir.EngineType.Activation,
                    mybir.EngineType.DVE)) as iv:
                body(iv)

    nc.compile()
    return nc


def host_prep(x, pos_table, Wq, bq, Wk, bk, Wv, bv, Wd, bd):
    B, S, D = x.shape
    f = np.float32
    bf = ml_dtypes.bfloat16
    shared = {
        "posT": np.ascontiguousarray(
            np.asarray(pos_table, dtype=f)[:S].T).astype(bf),
        "wqT": np.ascontiguousarray(
            (np.asarray(Wq, dtype=f).T / np.sqrt(np.float32(D)))
            @ np.asarray(Wk, dtype=f)).astype(bf),
        "wkT": np.ascontiguousarray(np.asarray(Wk, dtype=f).T).astype(bf),
        "wvT": np.ascontiguousarray(np.asarray(Wv, dtype=f).T).astype(bf),
        "wdT": np.ascontiguousarray(
            (np.asarray(Wd, dtype=f) @ np.asarray(Wv, dtype=f)).T).astype(bf),
        "bqs": np.zeros_like(np.asarray(bq, dtype=f)),
        "bk": np.asarray(bk, dtype=f),
        "bd": (np.asarray(bd, dtype=f)
               + np.asarray(Wd, dtype=f) @ np.asarray(bv, dtype=f)),
    }
    posT_f = np.asarray(pos_table, dtype=f)[:S].T
    in_maps = []
    for b in range(B):
        m = dict(shared)
        if "posfold" in OPTS:
            xpb = np.asarray(x[b], dtype=f) + posT_f.T
        else:
            xpb = np.asarray(x[b], dtype=f)
        m["xT"] = np.ascontiguousarray(xpb.T).astype(bf)
        m["xR"] = np.ascontiguousarray(xpb).astype(bf)
        m["rb"] = (xpb @ (np.asarray(Wk, dtype=f).T
                          @ (np.asarray(bq, dtype=f)
                             / np.sqrt(np.float32(D))))).astype(f)
        in_maps.append(m)
    return in_maps


_NC_CACHE = {}


def _get_nc(S, D, R=1):
    key = (S, D, R)
    if key not in _NC_CACHE:
        _NC_CACHE[key] = build_nc(S=S, D=D, R=R)
    return _NC_CACHE[key]


def kernel(x, pos_table, Wq, bq, Wk, bk, Wv, bv, Wd, bd):
    """Full inputs -> full output [B, S, D], computed on 8 NeuronCores."""
    x = np.asarray(x)
    B, S, D = x.shape
    assert B == 8, f"expected B=8, got {B}"
    nc = _get_nc(S, D)
    in_maps = host_prep(x, np.asarray(pos_table), np.asarray(Wq),
                        np.asarray(bq), np.asarray(Wk), np.asarray(bk),
                        np.asarray(Wv), np.asarray(bv), np.asarray(Wd),
                        np.asarray(bd))
    res = run_bass_kernel_spmd(nc, in_maps, core_ids=list(range(B)))
    return np.stack([np.asarray(res.results[b]["out"], dtype=np.float32)
                     for b in range(B)])

